# revision 37
# baseline (speedup 1.0000x reference)
"""Distributed 2-layer GCN (PyG GCNConv) + global mean pool + linear head
on 8 Trainium2 NeuronCores via Bass/Tile.

Strategy (dst-sharded graph parallel, edge-major DMA gather):
  - Nodes sharded contiguously across 8 cores (6250 each). Edges live on the
    core owning their dst node, grouped by (128-node dst block, table half),
    sorted by src within a cell for HBM locality.
  - Per layer, the message table  tab = (h @ W) * deg^-1/2  is built
    shard-locally in NODE-major layout ([node, 128] bf16 rows, features in
    cols 0:64), AllGathered into a shared HBM tensor [50000, 128].
  - Source-row fetch uses SWDGE dma_gather (gpsimd desc-gen, 16 DMA
    engines, 4 SWDGE queues round-robin so transfers overlap): 1024 edges
    per call, each edge pulls one 256B table row from HBM into SBUF
    EDGE-major ([128 edge slots, 8 tiles, 128 cols]). Indices are int16
    relative to a 25000-row table half; calls are per-half streams.
  - Segment-sum into dst nodes: per 128-edge tile one DVE is_equal builds
    Sel[e, j] = (dst_rel[e] == j) (batched 4 tiles per DVE instruction via
    a broadcast 3D AP); one TensorE matmul (lhsT=Sel, rhs=gathered
    rows[:, t, 0:64]) accumulates into a 3-deep PSUM ring of
    [128 nodes, 64 feat] dst blocks. Sel builds are input-independent, so
    the DVE runs ahead of the PE; no per-tile transpose or PSUM copy.
  - deg^-1/2 scaling, bias and ReLU fold into per-block init/flush ops; the
    flush also builds the next layer's table rows (h*dinv @ W2 via one PE
    transpose + matmul) and DMAs them node-major to the shard table.
  - Graph mean-pool: per-block matmul with a batch-id selection matrix into
    one PSUM accumulator [64 graphs, 64 feat | count col], AllReduced; the
    tiny linear head is computed redundantly on every core.
"""

import sys

sys.path.insert(0, "/opt/trn_rl_repo")

import numpy as np
import ml_dtypes

BF16 = ml_dtypes.bfloat16

import concourse.bass as bass
import concourse.bacc as bacc
import concourse.mybir as mybir
import concourse.tile as tile
from concourse.bass_utils import run_bass_kernel_spmd

F32 = mybir.dt.float32
BF = mybir.dt.bfloat16
I16 = mybir.dt.int16


class Cfg:
    def __init__(self, N=50000, E=600000, DIN=128, HID=64, NOUT=10, NG=64, NCORES=8):
        self.N, self.E, self.DIN, self.HID = N, E, DIN, HID
        self.NOUT, self.NG, self.NCORES = NOUT, NG, NCORES
        self.NPC = N // NCORES                    # nodes per core
        self.NB = (self.NPC + 127) // 128         # dst blocks per core
        self.NPCP = self.NB * 128
        self.NH = 2                               # table halves (int16 idx)
        self.HALF = N // 2
        self.SENT = 200.0                         # bf16-exact sentinel
        self.RING = 4                             # psum ring depth
        self.CTILES = 8                           # tiles per dma_gather call
        self.CIDX = self.CTILES * 128             # 1024 idxs per call (HW cap)
        self.GBUF = 6                             # gather bufs per half
        self.NS4 = 6                              # batched-sel buffer ring
        self.NSQ = 4                              # SWDGE queues


FULL = Cfg()


# ----------------------------------------------------------------------------
# Host-side schedule
# ----------------------------------------------------------------------------

def _schedule(cfg, src, dst):
    """Cells = (dst block b, table half h); tiles per cell = max over cores
    (SPMD-uniform). Stream order: (b asc, h asc, j asc). Gather calls pack
    CTILES consecutive same-half tiles."""
    C, NPC, NB, NH = cfg.NCORES, cfg.NPC, cfg.NB, cfg.NH
    order = np.argsort(dst, kind="stable")
    s_all = src[order]
    d_all = dst[order]
    cores = []
    for c in range(C):
        lo = np.searchsorted(d_all, c * NPC, side="left")
        hi = np.searchsorted(d_all, (c + 1) * NPC, side="left")
        s = s_all[lo:hi].astype(np.int64)
        d = (d_all[lo:hi] - c * NPC).astype(np.int64)
        key = (d // 128) * (NH * cfg.N) + (s // cfg.HALF) * cfg.N + s
        o2 = np.argsort(key, kind="stable")
        cores.append((s[o2], d[o2]))

    cnt = np.zeros((C, NB, NH), np.int64)
    start = np.zeros((C, NB, NH), np.int64)
    for c in range(C):
        s, d = cores[c]
        key = (d // 128) * NH + (s // cfg.HALF)
        bc = np.bincount(key, minlength=NB * NH).reshape(NB, NH)
        cnt[c] = bc
        start[c] = np.concatenate([[0], bc.reshape(-1).cumsum()[:-1]]).reshape(NB, NH)
    size = cnt.max(axis=0)               # [NB, NH] slots per cell

    tiles = []                            # stream order: (b, h, j)
    for b in range(NB):
        for h in range(NH):
            nt = -(-int(size[b, h]) // 128)
            for j in range(nt):
                tiles.append(dict(b=b, h=h, j=j))
    T = len(tiles)

    # per-half call assignment: call (h, k) covers the k-th run of CTILES
    # stream tiles of half h. tile -> (q, slot)
    half_tiles = {h: [t for t, m in enumerate(tiles) if m["h"] == h]
                  for h in range(NH)}
    calls = []                            # dicts: h, members
    tile_call = {}
    for h in range(NH):
        ts = half_tiles[h]
        for k in range(0, len(ts), cfg.CTILES):
            mem = ts[k:k + cfg.CTILES]
            q = len(calls)
            calls.append(dict(h=h, members=mem))
            for sl, t in enumerate(mem):
                tile_call[t] = (q, sl)
    NCALLS = len(calls)

    first_use = {q: min(c["members"]) for q, c in enumerate(calls)}
    last_use = {q: max(c["members"]) for q, c in enumerate(calls)}
    # buffer ring per half: call (h, k) -> buf k % GBUF; emit right after the
    # previous occupant's last tile so the WAR dep is already satisfied.
    emit_at = {}
    kh = {h: [] for h in range(cfg.NH)}
    for q, c in enumerate(calls):
        ks = kh[c["h"]]
        if len(ks) < cfg.GBUF:
            emit_at[q] = 0
        else:
            emit_at[q] = last_use[ks[-cfg.GBUF]] + 1
        ks.append(q)
    call_order = sorted(range(NCALLS), key=lambda q: (emit_at[q], first_use[q]))
    call_seq = {q: k for k, q in enumerate(call_order)}

    events = []
    emitted = 0
    for t, m in enumerate(tiles):
        b = m["b"]
        if t == 0 or tiles[t - 1]["b"] != b:
            if b >= 2:
                events.append(("flush", b - 2))
            events.append(("init", b))
        while emitted < NCALLS and emit_at[call_order[emitted]] <= t:
            events.append(("call", call_order[emitted]))
            emitted += 1
        events.append(("tile", t))
    while emitted < NCALLS:
        events.append(("call", call_order[emitted]))
        emitted += 1
    for b in range(max(0, NB - 2), NB):
        events.append(("flush", b))

    # per-core index + drel tables
    per_core = []
    for c in range(C):
        s, d = cores[c]
        idx16 = np.zeros((16, NCALLS * (cfg.CIDX // 16)), np.int16)
        drel = np.full((T, 128), cfg.SENT, np.float32)
        for t, m in enumerate(tiles):
            b, h, j = m["b"], m["h"], m["j"]
            q, sl = tile_call[t]
            kc = int(cnt[c, b, h])
            lo = 128 * j
            k = min(128, kc - lo)
            if k <= 0:
                continue
            e0 = int(start[c, b, h]) + lo
            rel = (s[e0:e0 + k] - h * cfg.HALF).astype(np.int16)
            i = sl * 128 + np.arange(k)
            idx16[i % 16, q * (cfg.CIDX // 16) + i // 16] = rel
            drel[t, :k] = (d[e0:e0 + k] - b * 128).astype(np.float32)
        idx128 = np.tile(idx16, (8, 1))
        per_core.append(dict(
            idx=np.ascontiguousarray(idx128),
            drel=np.ascontiguousarray(drel.T.astype(BF16)),
        ))

    return dict(events=events, tiles=tiles, calls=calls, tile_call=tile_call,
                call_seq=call_seq, T=T, NCALLS=NCALLS, per_core=per_core)


def _prepare(cfg, x, W1, b1, W2, b2, Wl, bl, edge_index, batch):
    src = np.asarray(edge_index[0], dtype=np.int64)
    dst = np.asarray(edge_index[1], dtype=np.int64)
    batch = np.asarray(batch, dtype=np.int64)
    x = np.asarray(x, dtype=np.float32)

    deg = np.bincount(dst, minlength=cfg.N).astype(np.float64) + 1.0
    dinv = (1.0 / np.sqrt(deg)).astype(np.float32)
    sqd = np.sqrt(deg).astype(np.float32)

    sch = _schedule(cfg, src, dst)

    iota = np.tile(np.arange(128, dtype=np.float32), (128, 1)).astype(BF16)
    idf = np.eye(128, dtype=np.float32)
    b1t = np.tile(np.asarray(b1, np.float32), (128, 1))
    b2t = np.tile(np.asarray(b2, np.float32), (128, 1))
    wlx = np.concatenate([np.asarray(Wl, np.float32),
                          np.asarray(bl, np.float32)[None, :]], 0).astype(BF16)

    in_maps = []
    for c in range(cfg.NCORES):
        lo, hi = c * cfg.NPC, (c + 1) * cfg.NPC
        xT = np.zeros((cfg.DIN, cfg.NPCP), np.float32)
        xT[:, :cfg.NPC] = x[lo:hi].T
        dloc = np.zeros((128, cfg.NB), np.float32)
        sloc = np.zeros((128, cfg.NB), np.float32)
        bat = np.full((128, cfg.NB), cfg.SENT, np.float32)
        dv, sq, bt = dinv[lo:hi], sqd[lo:hi], batch[lo:hi].astype(np.float32)
        for b in range(cfg.NB):
            r0, r1 = b * 128, min((b + 1) * 128, cfg.NPC)
            if r1 > r0:
                k = r1 - r0
                dloc[:k, b] = dv[r0:r1]
                sloc[:k, b] = sq[r0:r1]
                bat[:k, b] = bt[r0:r1]
        pc = sch["per_core"][c]
        in_maps.append({
            "xT": np.ascontiguousarray(xT),
            "idxg": pc["idx"],
            "drel": pc["drel"],
            "dinvc": np.ascontiguousarray(dloc),
            "sqdc": np.ascontiguousarray(sloc),
            "batchc": np.ascontiguousarray(bat.astype(BF16)),
            "b1t": b1t, "b2t": b2t,
            "w1": np.ascontiguousarray(np.asarray(W1, np.float32)),
            "w2b": np.ascontiguousarray(np.asarray(W2, np.float32)),
            "wlx": wlx,
            "iota": iota, "idf": idf,
        })
    return sch, in_maps


# ----------------------------------------------------------------------------
# Device program
# ----------------------------------------------------------------------------

def _build(cfg, sch):
    nc = bacc.Bacc(None, target_bir_lowering=False, num_swdge_queues=cfg.NSQ)
    NB, NPC, HID, NG = cfg.NB, cfg.NPC, cfg.HID, cfg.NG
    T, NCALLS = sch["T"], sch["NCALLS"]
    events, tiles, calls = sch["events"], sch["tiles"], sch["calls"]
    tile_call, call_seq = sch["tile_call"], sch["call_seq"]
    rep = [list(range(cfg.NCORES))]
    CI16 = cfg.CIDX // 16

    p = nc.declare_dram_parameter
    xT_d = p("xT", [cfg.DIN, cfg.NPCP], F32, isOutput=False)
    idx_d = p("idxg", [128, NCALLS * CI16], I16, isOutput=False)
    drel_d = p("drel", [128, T], BF, isOutput=False)
    dinv_d = p("dinvc", [128, NB], F32, isOutput=False)
    sqd_d = p("sqdc", [128, NB], F32, isOutput=False)
    bat_d = p("batchc", [128, NB], BF, isOutput=False)
    b1t_d = p("b1t", [128, HID], F32, isOutput=False)
    b2t_d = p("b2t", [128, HID], F32, isOutput=False)
    w1_d = p("w1", [cfg.DIN, HID], F32, isOutput=False)
    w2_d = p("w2b", [HID, HID], F32, isOutput=False)
    wlx_d = p("wlx", [HID + 1, cfg.NOUT], BF, isOutput=False)
    iota_d = p("iota", [128, 128], BF, isOutput=False)
    idf_d = p("idf", [128, 128], F32, isOutput=False)
    out_d = p("out", [NG, cfg.NOUT], F32, isOutput=True)

    t1sh = nc.dram_tensor("t1sh", [NPC, 128], BF)
    t2sh = nc.dram_tensor("t2sh", [NPC, 128], BF)
    t1full = nc.dram_tensor("t1full", [cfg.N, 128], BF, addr_space="Shared")
    t2full = nc.dram_tensor("t2full", [cfg.N, 128], BF, addr_space="Shared")
    pool_in = nc.dram_tensor("pool_in", [NG, HID + 1], F32)
    pool_out = nc.dram_tensor("pool_out", [NG, HID + 1], F32, addr_space="Shared")

    from contextlib import ExitStack
    ctx = ExitStack()
    sb = lambda name, shape, dt: ctx.enter_context(nc.sbuf_tensor(name, shape, dt))
    ps = lambda name, shape, dt: ctx.enter_context(nc.psum_tensor(name, shape, dt))

    with tile.TileContext(nc, num_cores=cfg.NCORES) as tc:
        idx_s = sb("idx_s", [128, NCALLS * CI16], I16)
        drel_s = sb("drel_s", [128, T], BF)
        dinv_s = sb("dinv_s", [128, NB], F32)
        sqd_s = sb("sqd_s", [128, NB], F32)
        bat_s = sb("bat_s", [128, NB], BF)
        b1t_s = sb("b1t_s", [128, HID], F32)
        b2t_s = sb("b2t_s", [128, HID], F32)
        w1_s = sb("w1_s", [cfg.DIN, HID], F32)
        xtb = [sb(f"xtb{i}", [cfg.DIN, 128], F32) for i in range(2)]
        w2_s = sb("w2_s", [HID, HID], F32)
        wlx_s = sb("wlx_s", [HID + 1, cfg.NOUT], BF)
        iota_s = sb("iota_s", [128, 128], BF)
        idf_s = sb("idf_s", [128, 128], F32)
        t1init = sb("t1init", [128, NB * HID], F32)
        t2init = sb("t2init", [128, NB * HID], F32)
        NGB = cfg.NH * cfg.GBUF
        gbuf = [sb(f"gbuf{i}", [128, cfg.CTILES * 128], BF) for i in range(NGB)]
        sel4 = [sb(f"sel4_{i}", [128, 4 * 128], BF) for i in range(cfg.NS4)]
        tmpv = [sb(f"tmpv{i}", [128, HID], F32) for i in range(2)]
        t1f = [sb(f"t1f{i}", [128, HID], F32) for i in range(2)]
        hdf = [sb(f"hdf{i}", [128, HID], F32) for i in range(2)]
        hdT = [sb(f"hdT{i}", [HID, 128], F32) for i in range(2)]
        tcb = [sb(f"tcb{i}", [128, 128], BF) for i in range(2)]
        h2e = [sb(f"h2e{i}", [128, HID + 1], BF) for i in range(2)]
        selg = [sb(f"selg{i}", [128, NG], BF) for i in range(2)]
        pool_s = sb("pool_s", [NG, HID + 1], F32)
        pool_r = sb("pool_r", [NG, HID + 1], F32)
        cnt_s = sb("cnt_s", [NG, 1], F32)
        rcp_s = sb("rcp_s", [NG, 1], F32)
        pooled_s = sb("pooled_s", [NG, HID], F32)
        pTx = sb("pTx", [HID + 1, NG], BF)
        out_s = sb("out_s", [NG, cfg.NOUT], F32)

        ring = [ps(f"ring{i}", [128, HID], F32) for i in range(cfg.RING)]
        ptA = ps("ptA0", [128, HID], F32)
        ptB = ps("ptB", [HID, 128], F32)
        pool_ps = ps("pool_ps", [NG, HID + 1], F32)

        gp, ve, sc, te, sy = nc.gpsimd, nc.vector, nc.scalar, nc.tensor, nc.sync

        for name_s, name_d in [(idx_s, idx_d), (drel_s, drel_d),
                               (dinv_s, dinv_d), (sqd_s, sqd_d), (bat_s, bat_d),
                               (b1t_s, b1t_d), (b2t_s, b2t_d), (w1_s, w1_d),
                               (w2_s, w2_d), (wlx_s, wlx_d), (iota_s, iota_d),
                               (idf_s, idf_d)]:
            sy.dma_start(out=name_s[:, :], in_=name_d[:, :])

        for i in range(2):
            ve.memset(tcb[i][:, HID:128], 0.0)

        # ---- phase A: table1 (node-major rows) + init1 ----------------------
        for b in range(NB):
            r0 = b * 128
            rows = min(128, NPC - r0)
            sy.dma_start(out=xtb[b % 2][:, :], in_=xT_d[:, r0:r0 + 128])
            nc.tensor.matmul(out=ptA[:, :], lhsT=xtb[b % 2][:, :],
                             rhs=w1_s[:, :], start=True, stop=True)
            sc.activation(t1f[b % 2][:, :], ptA[:, :],
                          mybir.ActivationFunctionType.Copy,
                          scale=dinv_s[:, b:b + 1])
            ve.tensor_mul(tmpv[b % 2][:, :], b1t_s[:, :],
                          sqd_s[:, b:b + 1].to_broadcast([128, HID]))
            ve.tensor_add(t1init[:, b * HID:(b + 1) * HID], tmpv[b % 2][:, :],
                          t1f[b % 2][:, :])
            ve.tensor_copy(tcb[b % 2][:, 0:HID], t1f[b % 2][:, :])
            sy.dma_start(out=t1sh[r0:r0 + rows, :], in_=tcb[b % 2][:rows, :])

        gp.collective_compute("AllGather", mybir.AluOpType.bypass,
                              replica_groups=rep, ins=[t1sh[:, :]],
                              outs=[t1full[:, :]])

        # last tile of each block (for matmul stop flags)
        last_tile = {}
        for t, m in enumerate(tiles):
            last_tile[m["b"]] = t

        # ---- message-passing layer ------------------------------------------
        # queue_num must track tile's global DMASW lane rotation (mod 8),
        # which continues across layers — use a global gather counter.
        gctr = [0]

        def layer(tfull, init_s, is_last):
            for ev, v in events:
                if ev == "call":
                    q = v
                    h = calls[q]["h"]
                    gb = gbuf[h * cfg.GBUF + _halfpos[q] % cfg.GBUF]
                    src = tfull[h * cfg.HALF:(h + 1) * cfg.HALF, :]
                    gp.dma_gather(
                        gb[:, :].rearrange("p (t e) -> p t e", e=128),
                        src,
                        idx_s[:, q * CI16:(q + 1) * CI16],
                        cfg.CIDX, cfg.CIDX, 128,
                        queue_num=(gctr[0] % 8) % cfg.NSQ,
                    )
                    gctr[0] += 1
                elif ev == "tile":
                    t = v
                    m = tiles[t]
                    q, sl = tile_call[t]
                    h = calls[q]["h"]
                    gb = gbuf[h * cfg.GBUF + _halfpos[q] % cfg.GBUF]
                    if t % 4 == 0:
                        n = min(4, T - t)
                        s4 = sel4[(t // 4) % cfg.NS4]
                        ve.tensor_tensor(
                            out=s4[:, 0:n * 128].rearrange(
                                "p (t e) -> p t e", e=128),
                            in0=drel_s[:, t:t + n].rearrange(
                                "p (t u) -> p t u", u=1).to_broadcast([128, n, 128]),
                            in1=iota_s[:, :].rearrange(
                                "p (u e) -> p u e", u=1).to_broadcast([128, n, 128]),
                            op=mybir.AluOpType.is_equal)
                    s4 = sel4[(t // 4) % cfg.NS4]
                    nc.tensor.matmul(
                        out=ring[m["b"] % cfg.RING][:, :],
                        lhsT=s4[:, (t % 4) * 128:(t % 4 + 1) * 128],
                        rhs=gb[:, sl * 128:sl * 128 + HID],
                        start=False, stop=(last_tile[m["b"]] == t),
                        skip_group_check=True)
                elif ev == "init":
                    b = v
                    nc.tensor.matmul(out=ring[b % cfg.RING][:, :],
                                     lhsT=idf_s[:, :],
                                     rhs=init_s[:, b * HID:(b + 1) * HID],
                                     start=True, stop=(b not in last_tile),
                                     skip_group_check=True)
                else:  # flush
                    b = v
                    rg = ring[b % cfg.RING]
                    r0 = b * 128
                    rows = min(128, NPC - r0)
                    if not is_last:
                        sc.activation(hdf[b % 2][:, :], rg[:, :],
                                      mybir.ActivationFunctionType.Relu,
                                      scale=dinv_s[:, b:b + 1])
                        sc.activation(hdf[b % 2][:, :], hdf[b % 2][:, :],
                                      mybir.ActivationFunctionType.Copy,
                                      scale=dinv_s[:, b:b + 1])
                        nc.tensor.matmul(out=ptB[:, :], lhsT=hdf[b % 2][:, :],
                                         rhs=idf_s[:, :], is_transpose=True)
                        ve.tensor_copy(hdT[b % 2][:, :], ptB[:, :])
                        nc.tensor.matmul(out=ptA[:, :], lhsT=hdT[b % 2][:, :],
                                         rhs=w2_s[:, :], start=True, stop=True)
                        ve.tensor_mul(tmpv[b % 2][:, :], b2t_s[:, :],
                                      sqd_s[:, b:b + 1].to_broadcast([128, HID]))
                        ve.tensor_add(t2init[:, b * HID:(b + 1) * HID],
                                      tmpv[b % 2][:, :], ptA[:, :])
                        ve.tensor_copy(tcb[b % 2][:, 0:HID], ptA[:, :])
                        sy.dma_start(out=t2sh[r0:r0 + rows, :],
                                     in_=tcb[b % 2][:rows, :])
                    else:
                        hh = h2e[b % 2]
                        ve.memset(hh[:, HID:HID + 1], 1.0)
                        sc.activation(hh[:, 0:HID], rg[:, :],
                                      mybir.ActivationFunctionType.Relu,
                                      scale=dinv_s[:, b:b + 1])
                        ve.tensor_tensor(out=selg[b % 2][:, :],
                                         in0=bat_s[:, b:b + 1].to_broadcast([128, NG]),
                                         in1=iota_s[:, 0:NG],
                                         op=mybir.AluOpType.is_equal)
                        nc.tensor.matmul(out=pool_ps[:, :], lhsT=selg[b % 2][:, :],
                                         rhs=hh[:, :], start=(b == 0),
                                         stop=(b == NB - 1), skip_group_check=True)

        # call q -> per-half round-robin buffer position
        _halfpos = {}
        _seen = {0: 0, 1: 0}
        for q, c in enumerate(calls):
            _halfpos[q] = _seen[c["h"]]
            _seen[c["h"]] += 1

        layer(t1full, t1init, is_last=False)
        gp.collective_compute("AllGather", mybir.AluOpType.bypass,
                              replica_groups=rep, ins=[t2sh[:, :]],
                              outs=[t2full[:, :]])
        layer(t2full, t2init, is_last=True)

        # ---- pooling finale --------------------------------------------------
        ve.tensor_copy(pool_s[:, :], pool_ps[:, :])
        sy.dma_start(out=pool_in[:, :], in_=pool_s[:, :])
        gp.collective_compute("AllReduce", mybir.AluOpType.add,
                              replica_groups=rep, ins=[pool_in[:, :]],
                              outs=[pool_out[:, :]])
        sy.dma_start(out=pool_r[:, :], in_=pool_out[:, :])
        ve.tensor_scalar_max(cnt_s[:, :], pool_r[:, HID:HID + 1], 1.0)
        ve.reciprocal(rcp_s[:, :], cnt_s[:, :])
        ve.tensor_mul(pooled_s[:, :], pool_r[:, 0:HID],
                      rcp_s[:, :].to_broadcast([NG, HID]))
        nc.tensor.matmul(out=ptB[:, 0:NG], lhsT=pooled_s[:, :],
                         rhs=idf_s[0:NG, 0:NG], is_transpose=True)
        ve.memset(pTx[HID:HID + 1, :], 1.0)
        ve.tensor_copy(pTx[0:HID, :], ptB[0:HID, 0:NG])
        nc.tensor.matmul(out=ptA[0:NG, 0:cfg.NOUT], lhsT=pTx[:, :],
                         rhs=wlx_s[:, :], start=True, stop=True)
        ve.tensor_copy(out_s[:, :], ptA[0:NG, 0:cfg.NOUT])
        sy.dma_start(out=out_d[:, :], in_=out_s[:, :])

    # ctx deliberately left open (const APs interleave with our stack entries)
    nc.finalize()
    return nc


# ----------------------------------------------------------------------------
# Entry
# ----------------------------------------------------------------------------

def run_gcn(cfg, x, W1, b1, W2, b2, Wl, bl, edge_index, batch, trace=False):
    sch, in_maps = _prepare(cfg, x, W1, b1, W2, b2, Wl, bl, edge_index, batch)
    nc = _build(cfg, sch)
    res = run_bass_kernel_spmd(nc, in_maps, core_ids=list(range(cfg.NCORES)),
                               trace=trace)
    return np.asarray(res.results[0]["out"], dtype=np.float32), res


def kernel(**inputs):
    out, _ = run_gcn(
        FULL,
        inputs["x"], inputs["W1"], inputs["b1"], inputs["W2"], inputs["b2"],
        inputs["Wl"], inputs["bl"], inputs["edge_index"], inputs["batch"],
    )
    return out


# revision 39
# speedup vs baseline: 1.0776x; 1.0776x over previous
"""Distributed 2-layer GCN (PyG GCNConv) + global mean pool + linear head
on 8 Trainium2 NeuronCores via Bass/Tile.

Strategy (dst-sharded graph parallel, edge-major DMA gather):
  - Nodes sharded contiguously across 8 cores (6250 each). Edges live on the
    core owning their dst node, grouped by (128-node dst block, table half),
    sorted by src within a cell for HBM locality.
  - Per layer, the message table  tab = (h @ W) * deg^-1/2  is built
    shard-locally in NODE-major layout ([node, 128] bf16 rows, features in
    cols 0:64), AllGathered into a shared HBM tensor [50000, 128].
  - Source-row fetch uses SWDGE dma_gather (gpsimd desc-gen, 16 DMA
    engines, 4 SWDGE queues round-robin so transfers overlap): 1024 edges
    per call, each edge pulls one 256B table row from HBM into SBUF
    EDGE-major ([128 edge slots, 8 tiles, 128 cols]). Indices are int16
    relative to a 25000-row table half; calls are per-half streams.
  - Segment-sum into dst nodes: per 128-edge tile one DVE is_equal builds
    Sel[e, j] = (dst_rel[e] == j) (batched 4 tiles per DVE instruction via
    a broadcast 3D AP); one TensorE matmul (lhsT=Sel, rhs=gathered
    rows[:, t, 0:64]) accumulates into a 3-deep PSUM ring of
    [128 nodes, 64 feat] dst blocks. Sel builds are input-independent, so
    the DVE runs ahead of the PE; no per-tile transpose or PSUM copy.
  - deg^-1/2 scaling, bias and ReLU fold into per-block init/flush ops; the
    flush also builds the next layer's table rows (h*dinv @ W2 via one PE
    transpose + matmul) and DMAs them node-major to the shard table.
  - Graph mean-pool: per-block matmul with a batch-id selection matrix into
    one PSUM accumulator [64 graphs, 64 feat | count col], AllReduced; the
    tiny linear head is computed redundantly on every core.
"""

import sys

sys.path.insert(0, "/opt/trn_rl_repo")

import numpy as np
import ml_dtypes

BF16 = ml_dtypes.bfloat16

import concourse.bass as bass
import concourse.bacc as bacc
import concourse.mybir as mybir
import concourse.tile as tile
from concourse.bass_utils import run_bass_kernel_spmd

F32 = mybir.dt.float32
BF = mybir.dt.bfloat16
I16 = mybir.dt.int16


class Cfg:
    def __init__(self, N=50000, E=600000, DIN=128, HID=64, NOUT=10, NG=64, NCORES=8):
        self.N, self.E, self.DIN, self.HID = N, E, DIN, HID
        self.NOUT, self.NG, self.NCORES = NOUT, NG, NCORES
        self.NPC = N // NCORES                    # nodes per core
        self.NB = (self.NPC + 127) // 128         # dst blocks per core
        self.NPCP = self.NB * 128
        self.NH = 2                               # table halves (int16 idx)
        self.HALF = N // 2
        self.SENT = 200.0                         # bf16-exact sentinel
        self.RING = 3                             # psum ring depth
        self.CTILES = 8                           # tiles per dma_gather call
        self.CIDX = self.CTILES * 128             # 1024 idxs per call (HW cap)
        self.GBUF = 6                             # gather bufs per half
        self.NS4 = 6                              # batched-sel buffer ring
        self.NSQ = 4                              # SWDGE queues


FULL = Cfg()


# ----------------------------------------------------------------------------
# Host-side schedule
# ----------------------------------------------------------------------------

def _schedule(cfg, src, dst):
    """Cells = (dst block b, table half h); tiles per cell = max over cores
    (SPMD-uniform). Stream order: (b asc, h asc, j asc). Gather calls pack
    CTILES consecutive same-half tiles."""
    C, NPC, NB, NH = cfg.NCORES, cfg.NPC, cfg.NB, cfg.NH
    order = np.argsort(dst, kind="stable")
    s_all = src[order]
    d_all = dst[order]
    cores = []
    for c in range(C):
        lo = np.searchsorted(d_all, c * NPC, side="left")
        hi = np.searchsorted(d_all, (c + 1) * NPC, side="left")
        s = s_all[lo:hi].astype(np.int64)
        d = (d_all[lo:hi] - c * NPC).astype(np.int64)
        key = (d // 128) * (NH * cfg.N) + (s // cfg.HALF) * cfg.N + s
        o2 = np.argsort(key, kind="stable")
        cores.append((s[o2], d[o2]))

    cnt = np.zeros((C, NB, NH), np.int64)
    start = np.zeros((C, NB, NH), np.int64)
    for c in range(C):
        s, d = cores[c]
        key = (d // 128) * NH + (s // cfg.HALF)
        bc = np.bincount(key, minlength=NB * NH).reshape(NB, NH)
        cnt[c] = bc
        start[c] = np.concatenate([[0], bc.reshape(-1).cumsum()[:-1]]).reshape(NB, NH)
    size = cnt.max(axis=0)               # [NB, NH] slots per cell

    tiles = []                            # stream order: (b, h, j)
    for b in range(NB):
        for h in range(NH):
            nt = -(-int(size[b, h]) // 128)
            for j in range(nt):
                tiles.append(dict(b=b, h=h, j=j))
    T = len(tiles)

    # per-half call assignment: call (h, k) covers the k-th run of CTILES
    # stream tiles of half h. tile -> (q, slot)
    half_tiles = {h: [t for t, m in enumerate(tiles) if m["h"] == h]
                  for h in range(NH)}
    calls = []                            # dicts: h, members
    tile_call = {}
    for h in range(NH):
        ts = half_tiles[h]
        for k in range(0, len(ts), cfg.CTILES):
            mem = ts[k:k + cfg.CTILES]
            q = len(calls)
            calls.append(dict(h=h, members=mem))
            for sl, t in enumerate(mem):
                tile_call[t] = (q, sl)
    NCALLS = len(calls)

    first_use = {q: min(c["members"]) for q, c in enumerate(calls)}
    last_use = {q: max(c["members"]) for q, c in enumerate(calls)}
    # buffer ring per half: call (h, k) -> buf k % GBUF; emit right after the
    # previous occupant's last tile so the WAR dep is already satisfied.
    emit_at = {}
    kh = {h: [] for h in range(cfg.NH)}
    for q, c in enumerate(calls):
        ks = kh[c["h"]]
        if len(ks) < cfg.GBUF:
            emit_at[q] = 0
        else:
            emit_at[q] = last_use[ks[-cfg.GBUF]] + 1
        ks.append(q)
    call_order = sorted(range(NCALLS), key=lambda q: (emit_at[q], first_use[q]))
    call_seq = {q: k for k, q in enumerate(call_order)}

    events = []
    emitted = 0
    for t, m in enumerate(tiles):
        b = m["b"]
        if t == 0 or tiles[t - 1]["b"] != b:
            if b >= 2:
                events.append(("flush", b - 2))
            events.append(("init", b))
        while emitted < NCALLS and emit_at[call_order[emitted]] <= t:
            events.append(("call", call_order[emitted]))
            emitted += 1
        events.append(("tile", t))
    while emitted < NCALLS:
        events.append(("call", call_order[emitted]))
        emitted += 1
    for b in range(max(0, NB - 2), NB):
        events.append(("flush", b))

    # per-core index + drel tables
    per_core = []
    for c in range(C):
        s, d = cores[c]
        idx16 = np.zeros((16, NCALLS * (cfg.CIDX // 16)), np.int16)
        drel = np.full((T, 128), cfg.SENT, np.float32)
        for t, m in enumerate(tiles):
            b, h, j = m["b"], m["h"], m["j"]
            q, sl = tile_call[t]
            kc = int(cnt[c, b, h])
            lo = 128 * j
            k = min(128, kc - lo)
            if k <= 0:
                continue
            e0 = int(start[c, b, h]) + lo
            rel = (s[e0:e0 + k] - h * cfg.HALF).astype(np.int16)
            i = sl * 128 + np.arange(k)
            idx16[i % 16, q * (cfg.CIDX // 16) + i // 16] = rel
            drel[t, :k] = (d[e0:e0 + k] - b * 128).astype(np.float32)
        idx128 = np.tile(idx16, (8, 1))
        per_core.append(dict(
            idx=np.ascontiguousarray(idx128),
            drel=np.ascontiguousarray(drel.T.astype(BF16)),
        ))

    return dict(events=events, tiles=tiles, calls=calls, tile_call=tile_call,
                call_seq=call_seq, T=T, NCALLS=NCALLS, per_core=per_core)


def _prepare(cfg, x, W1, b1, W2, b2, Wl, bl, edge_index, batch):
    src = np.asarray(edge_index[0], dtype=np.int64)
    dst = np.asarray(edge_index[1], dtype=np.int64)
    batch = np.asarray(batch, dtype=np.int64)
    x = np.asarray(x, dtype=np.float32)

    deg = np.bincount(dst, minlength=cfg.N).astype(np.float64) + 1.0
    dinv = (1.0 / np.sqrt(deg)).astype(np.float32)
    sqd = np.sqrt(deg).astype(np.float32)

    sch = _schedule(cfg, src, dst)

    iota = np.tile(np.arange(128, dtype=np.float32), (128, 1)).astype(BF16)
    idf = np.eye(128, dtype=np.float32)
    b1t = np.tile(np.asarray(b1, np.float32), (128, 1))
    b2t = np.tile(np.asarray(b2, np.float32), (128, 1))
    wlx = np.concatenate([np.asarray(Wl, np.float32),
                          np.asarray(bl, np.float32)[None, :]], 0).astype(BF16)

    in_maps = []
    for c in range(cfg.NCORES):
        lo, hi = c * cfg.NPC, (c + 1) * cfg.NPC
        xT = np.zeros((cfg.DIN, cfg.NPCP), np.float32)
        xT[:, :cfg.NPC] = x[lo:hi].T
        dloc = np.zeros((128, cfg.NB), np.float32)
        sloc = np.zeros((128, cfg.NB), np.float32)
        bat = np.full((128, cfg.NB), cfg.SENT, np.float32)
        dv, sq, bt = dinv[lo:hi], sqd[lo:hi], batch[lo:hi].astype(np.float32)
        for b in range(cfg.NB):
            r0, r1 = b * 128, min((b + 1) * 128, cfg.NPC)
            if r1 > r0:
                k = r1 - r0
                dloc[:k, b] = dv[r0:r1]
                sloc[:k, b] = sq[r0:r1]
                bat[:k, b] = bt[r0:r1]
        pc = sch["per_core"][c]
        in_maps.append({
            "xT": np.ascontiguousarray(xT),
            "idxg": pc["idx"],
            "drel": pc["drel"],
            "dinvc": np.ascontiguousarray(dloc),
            "sqdc": np.ascontiguousarray(sloc),
            "batchc": np.ascontiguousarray(bat.astype(BF16)),
            "b1t": b1t, "b2t": b2t,
            "w1": np.ascontiguousarray(np.asarray(W1, np.float32)),
            "w2b": np.ascontiguousarray(np.asarray(W2, np.float32)),
            "wlx": wlx,
            "iota": iota, "idf": idf,
        })
    return sch, in_maps


# ----------------------------------------------------------------------------
# Device program
# ----------------------------------------------------------------------------

def _build(cfg, sch):
    nc = bacc.Bacc(None, target_bir_lowering=False, num_swdge_queues=cfg.NSQ)
    NB, NPC, HID, NG = cfg.NB, cfg.NPC, cfg.HID, cfg.NG
    T, NCALLS = sch["T"], sch["NCALLS"]
    events, tiles, calls = sch["events"], sch["tiles"], sch["calls"]
    tile_call, call_seq = sch["tile_call"], sch["call_seq"]
    rep = [list(range(cfg.NCORES))]
    CI16 = cfg.CIDX // 16

    p = nc.declare_dram_parameter
    xT_d = p("xT", [cfg.DIN, cfg.NPCP], F32, isOutput=False)
    idx_d = p("idxg", [128, NCALLS * CI16], I16, isOutput=False)
    drel_d = p("drel", [128, T], BF, isOutput=False)
    dinv_d = p("dinvc", [128, NB], F32, isOutput=False)
    sqd_d = p("sqdc", [128, NB], F32, isOutput=False)
    bat_d = p("batchc", [128, NB], BF, isOutput=False)
    b1t_d = p("b1t", [128, HID], F32, isOutput=False)
    b2t_d = p("b2t", [128, HID], F32, isOutput=False)
    w1_d = p("w1", [cfg.DIN, HID], F32, isOutput=False)
    w2_d = p("w2b", [HID, HID], F32, isOutput=False)
    wlx_d = p("wlx", [HID + 1, cfg.NOUT], BF, isOutput=False)
    iota_d = p("iota", [128, 128], BF, isOutput=False)
    idf_d = p("idf", [128, 128], F32, isOutput=False)
    out_d = p("out", [NG, cfg.NOUT], F32, isOutput=True)

    t1sh = nc.dram_tensor("t1sh", [NPC, 128], BF)
    t2sh = nc.dram_tensor("t2sh", [NPC, 128], BF)
    t1full = nc.dram_tensor("t1full", [cfg.N, 128], BF, addr_space="Shared")
    t2full = nc.dram_tensor("t2full", [cfg.N, 128], BF, addr_space="Shared")
    pool_in = nc.dram_tensor("pool_in", [NG, HID + 1], F32)
    pool_out = nc.dram_tensor("pool_out", [NG, HID + 1], F32, addr_space="Shared")

    from contextlib import ExitStack
    ctx = ExitStack()
    sb = lambda name, shape, dt: ctx.enter_context(nc.sbuf_tensor(name, shape, dt))
    ps = lambda name, shape, dt: ctx.enter_context(nc.psum_tensor(name, shape, dt))

    with tile.TileContext(nc, num_cores=cfg.NCORES) as tc:
        idx_s = sb("idx_s", [128, NCALLS * CI16], I16)
        drel_s = sb("drel_s", [128, T], BF)
        dinv_s = sb("dinv_s", [128, NB], F32)
        sqd_s = sb("sqd_s", [128, NB], F32)
        bat_s = sb("bat_s", [128, NB], BF)
        b1t_s = sb("b1t_s", [128, HID], F32)
        b2t_s = sb("b2t_s", [128, HID], F32)
        w1_s = sb("w1_s", [cfg.DIN, HID], F32)
        xtb = [sb(f"xtb{i}", [cfg.DIN, 128], F32) for i in range(2)]
        w2_s = sb("w2_s", [HID, HID], F32)
        wlx_s = sb("wlx_s", [HID + 1, cfg.NOUT], BF)
        iota_s = sb("iota_s", [128, 128], BF)
        idf_s = sb("idf_s", [128, 128], F32)
        idfb_s = sb("idfb_s", [128, 128], BF)
        t1init = sb("t1init", [128, NB * HID], BF)
        t2init = sb("t2init", [128, NB * HID], BF)
        NGB = cfg.NH * cfg.GBUF
        gbuf = [sb(f"gbuf{i}", [128, cfg.CTILES * 128], BF) for i in range(NGB)]
        sel4 = [sb(f"sel4_{i}", [128, 4 * 128], BF) for i in range(cfg.NS4)]
        tmpv = [sb(f"tmpv{i}", [128, HID], F32) for i in range(2)]
        t1f = [sb(f"t1f{i}", [128, HID], F32) for i in range(2)]
        hdf = [sb(f"hdf{i}", [128, HID], F32) for i in range(2)]
        hdT = [sb(f"hdT{i}", [HID, 128], F32) for i in range(2)]
        tcb = [sb(f"tcb{i}", [128, 128], BF) for i in range(2)]
        h2e = [sb(f"h2e{i}", [128, HID + 1], BF) for i in range(2)]
        selg = [sb(f"selg{i}", [128, NG], BF) for i in range(2)]
        pool_s = sb("pool_s", [NG, HID + 1], F32)
        pool_r = sb("pool_r", [NG, HID + 1], F32)
        cnt_s = sb("cnt_s", [NG, 1], F32)
        rcp_s = sb("rcp_s", [NG, 1], F32)
        pooled_s = sb("pooled_s", [NG, HID], F32)
        pTx = sb("pTx", [HID + 1, NG], BF)
        out_s = sb("out_s", [NG, cfg.NOUT], F32)

        ring = [ps(f"ring{i}", [128, HID], F32) for i in range(cfg.RING)]
        ptA = ps("ptA0", [128, HID], F32)
        ptB = ps("ptB", [HID, 128], F32)
        pool_ps = ps("pool_ps", [NG, HID + 1], F32)

        gp, ve, sc, te, sy = nc.gpsimd, nc.vector, nc.scalar, nc.tensor, nc.sync

        for name_s, name_d in [(idx_s, idx_d), (drel_s, drel_d),
                               (dinv_s, dinv_d), (sqd_s, sqd_d), (bat_s, bat_d),
                               (b1t_s, b1t_d), (b2t_s, b2t_d), (w1_s, w1_d),
                               (w2_s, w2_d), (wlx_s, wlx_d), (iota_s, iota_d),
                               (idf_s, idf_d)]:
            sy.dma_start(out=name_s[:, :], in_=name_d[:, :])

        for i in range(2):
            ve.memset(tcb[i][:, HID:128], 0.0)

        ve.tensor_copy(idfb_s[:, :], idf_s[:, :])

        # ---- phase A: table1 (node-major rows) + init1 ----------------------
        for b in range(NB):
            r0 = b * 128
            rows = min(128, NPC - r0)
            sy.dma_start(out=xtb[b % 2][:, :], in_=xT_d[:, r0:r0 + 128])
            nc.tensor.matmul(out=ptA[:, :], lhsT=xtb[b % 2][:, :],
                             rhs=w1_s[:, :], start=True, stop=True)
            sc.activation(t1f[b % 2][:, :], ptA[:, :],
                          mybir.ActivationFunctionType.Copy,
                          scale=dinv_s[:, b:b + 1])
            ve.tensor_mul(tmpv[b % 2][:, :], b1t_s[:, :],
                          sqd_s[:, b:b + 1].to_broadcast([128, HID]))
            ve.tensor_add(t1init[:, b * HID:(b + 1) * HID], tmpv[b % 2][:, :],
                          t1f[b % 2][:, :])
            ve.tensor_copy(tcb[b % 2][:, 0:HID], t1f[b % 2][:, :])
            sy.dma_start(out=t1sh[r0:r0 + rows, :], in_=tcb[b % 2][:rows, :])

        gp.collective_compute("AllGather", mybir.AluOpType.bypass,
                              replica_groups=rep, ins=[t1sh[:, :]],
                              outs=[t1full[:, :]])

        # last tile of each block (for matmul stop flags)
        last_tile = {}
        for t, m in enumerate(tiles):
            last_tile[m["b"]] = t

        # ---- message-passing layer ------------------------------------------
        # queue_num must track tile's global DMASW lane rotation (mod 8),
        # which continues across layers — use a global gather counter.
        gctr = [0]

        def layer(tfull, init_s, is_last):
            for ev, v in events:
                if ev == "call":
                    q = v
                    h = calls[q]["h"]
                    gb = gbuf[h * cfg.GBUF + _halfpos[q] % cfg.GBUF]
                    src = tfull[h * cfg.HALF:(h + 1) * cfg.HALF, :]
                    gp.dma_gather(
                        gb[:, :].rearrange("p (t e) -> p t e", e=128),
                        src,
                        idx_s[:, q * CI16:(q + 1) * CI16],
                        cfg.CIDX, cfg.CIDX, 128,
                        queue_num=(gctr[0] % 8) % cfg.NSQ,
                    )
                    gctr[0] += 1
                elif ev == "tile":
                    t = v
                    m = tiles[t]
                    q, sl = tile_call[t]
                    h = calls[q]["h"]
                    gb = gbuf[h * cfg.GBUF + _halfpos[q] % cfg.GBUF]
                    if t % 4 == 0:
                        n = min(4, T - t)
                        s4 = sel4[(t // 4) % cfg.NS4]
                        ve.tensor_tensor(
                            out=s4[:, 0:n * 128].rearrange(
                                "p (t e) -> p t e", e=128),
                            in0=drel_s[:, t:t + n].rearrange(
                                "p (t u) -> p t u", u=1).to_broadcast([128, n, 128]),
                            in1=iota_s[:, :].rearrange(
                                "p (u e) -> p u e", u=1).to_broadcast([128, n, 128]),
                            op=mybir.AluOpType.is_equal)
                    s4 = sel4[(t // 4) % cfg.NS4]
                    nc.tensor.matmul(
                        out=ring[m["b"] % cfg.RING][:, :],
                        lhsT=s4[:, (t % 4) * 128:(t % 4 + 1) * 128],
                        rhs=gb[:, sl * 128:sl * 128 + HID],
                        start=False, stop=(last_tile[m["b"]] == t),
                        skip_group_check=True)
                elif ev == "init":
                    b = v
                    nc.tensor.matmul(out=ring[b % cfg.RING][:, :],
                                     lhsT=idfb_s[:, :],
                                     rhs=init_s[:, b * HID:(b + 1) * HID],
                                     start=True, stop=(b not in last_tile),
                                     skip_group_check=True)
                else:  # flush
                    b = v
                    rg = ring[b % cfg.RING]
                    r0 = b * 128
                    rows = min(128, NPC - r0)
                    if not is_last:
                        sc.activation(hdf[b % 2][:, :], rg[:, :],
                                      mybir.ActivationFunctionType.Relu,
                                      scale=dinv_s[:, b:b + 1])
                        sc.activation(hdf[b % 2][:, :], hdf[b % 2][:, :],
                                      mybir.ActivationFunctionType.Copy,
                                      scale=dinv_s[:, b:b + 1])
                        nc.tensor.matmul(out=ptB[:, :], lhsT=hdf[b % 2][:, :],
                                         rhs=idf_s[:, :], is_transpose=True)
                        ve.tensor_copy(hdT[b % 2][:, :], ptB[:, :])
                        nc.tensor.matmul(out=ptA[:, :], lhsT=hdT[b % 2][:, :],
                                         rhs=w2_s[:, :], start=True, stop=True)
                        ve.tensor_mul(tmpv[b % 2][:, :], b2t_s[:, :],
                                      sqd_s[:, b:b + 1].to_broadcast([128, HID]))
                        ve.tensor_add(t2init[:, b * HID:(b + 1) * HID],
                                      tmpv[b % 2][:, :], ptA[:, :])
                        ve.tensor_copy(tcb[b % 2][:, 0:HID], ptA[:, :])
                        sy.dma_start(out=t2sh[r0:r0 + rows, :],
                                     in_=tcb[b % 2][:rows, :])
                    else:
                        hh = h2e[b % 2]
                        ve.memset(hh[:, HID:HID + 1], 1.0)
                        sc.activation(hh[:, 0:HID], rg[:, :],
                                      mybir.ActivationFunctionType.Relu,
                                      scale=dinv_s[:, b:b + 1])
                        ve.tensor_tensor(out=selg[b % 2][:, :],
                                         in0=bat_s[:, b:b + 1].to_broadcast([128, NG]),
                                         in1=iota_s[:, 0:NG],
                                         op=mybir.AluOpType.is_equal)
                        nc.tensor.matmul(out=pool_ps[:, :], lhsT=selg[b % 2][:, :],
                                         rhs=hh[:, :], start=(b == 0),
                                         stop=(b == NB - 1), skip_group_check=True)

        # call q -> per-half round-robin buffer position
        _halfpos = {}
        _seen = {0: 0, 1: 0}
        for q, c in enumerate(calls):
            _halfpos[q] = _seen[c["h"]]
            _seen[c["h"]] += 1

        layer(t1full, t1init, is_last=False)
        gp.collective_compute("AllGather", mybir.AluOpType.bypass,
                              replica_groups=rep, ins=[t2sh[:, :]],
                              outs=[t2full[:, :]])
        layer(t2full, t2init, is_last=True)

        # ---- pooling finale --------------------------------------------------
        ve.tensor_copy(pool_s[:, :], pool_ps[:, :])
        sy.dma_start(out=pool_in[:, :], in_=pool_s[:, :])
        gp.collective_compute("AllReduce", mybir.AluOpType.add,
                              replica_groups=rep, ins=[pool_in[:, :]],
                              outs=[pool_out[:, :]])
        sy.dma_start(out=pool_r[:, :], in_=pool_out[:, :])
        ve.tensor_scalar_max(cnt_s[:, :], pool_r[:, HID:HID + 1], 1.0)
        ve.reciprocal(rcp_s[:, :], cnt_s[:, :])
        ve.tensor_mul(pooled_s[:, :], pool_r[:, 0:HID],
                      rcp_s[:, :].to_broadcast([NG, HID]))
        nc.tensor.matmul(out=ptB[:, 0:NG], lhsT=pooled_s[:, :],
                         rhs=idf_s[0:NG, 0:NG], is_transpose=True)
        ve.memset(pTx[HID:HID + 1, :], 1.0)
        ve.tensor_copy(pTx[0:HID, :], ptB[0:HID, 0:NG])
        nc.tensor.matmul(out=ptA[0:NG, 0:cfg.NOUT], lhsT=pTx[:, :],
                         rhs=wlx_s[:, :], start=True, stop=True)
        ve.tensor_copy(out_s[:, :], ptA[0:NG, 0:cfg.NOUT])
        sy.dma_start(out=out_d[:, :], in_=out_s[:, :])

    # ctx deliberately left open (const APs interleave with our stack entries)
    nc.finalize()
    return nc


# ----------------------------------------------------------------------------
# Entry
# ----------------------------------------------------------------------------

def run_gcn(cfg, x, W1, b1, W2, b2, Wl, bl, edge_index, batch, trace=False):
    sch, in_maps = _prepare(cfg, x, W1, b1, W2, b2, Wl, bl, edge_index, batch)
    nc = _build(cfg, sch)
    res = run_bass_kernel_spmd(nc, in_maps, core_ids=list(range(cfg.NCORES)),
                               trace=trace)
    return np.asarray(res.results[0]["out"], dtype=np.float32), res


def kernel(**inputs):
    out, _ = run_gcn(
        FULL,
        inputs["x"], inputs["W1"], inputs["b1"], inputs["W2"], inputs["b2"],
        inputs["Wl"], inputs["bl"], inputs["edge_index"], inputs["batch"],
    )
    return out
